# revision 11
# baseline (speedup 1.0000x reference)
"""Trainium2 Bass kernel for the DeletionChannel problem.

Contract: kernel(**inputs) takes the FULL unsharded inputs
  message: [64, 4096, 128] f32, noise: [64, 4096] f32, message_length: [64] i64
and returns the full output [64, 4096, 128] f32.

Strategy: pure data parallel over batch — 8 rows per NeuronCore, 8 cores.
Per row: compute the deletion mask and the stable-compaction destination
rank of every source row with DVE prefix sums (+ a tiny block-diagonal
PE matmul for cross-partition offsets), overwrite deleted rows with the
eos one-hot in SBUF, then scatter all 4096 rows (512B each) to their
output positions with dma_scatter_add (custom SWDGE scatter,
out[idx] += src) onto the zero-initialized output buffer.

Perf structure (v2): reads (HBM->SBUF loads) and writes (scatter) each
cap at ~105-110 GB/s per core but run on independent DMA directions, so
the kernel software-pipelines at quarter-row granularity: load 512KB
quarter -> DVE mask -> 1024-token scatter, across 8 rows x 4 quarters,
keeping both directions saturated from ~10us onward.

Token mapping (HW-verified): scatter token t reads SBUF [t%128, t//128]
and writes output row idx[t], where idx[t] sits at [t%16, t//16] of a
[16, 256] int16 tile (replicated across the 8 16-partition groups). We
load message row R(t) = 256*(t%16) + 32*((t%128)//16) + t//128 into
slot [t%128, t//128] (16KB contiguous per partition), compute ranks in
a [16, 256] layout (replicated x8 on 128 partitions) where position
[q, m] owns row l = 256q + m, and emit idx via one strided DVE copy.
"""

import sys
from contextlib import ExitStack

sys.path.insert(0, "/opt/trn_rl_repo")

import numpy as np

import concourse.bacc as bacc
import concourse.bass as bass
import concourse.mybir as mybir
import concourse.tile as tile
from concourse.bass_utils import run_bass_kernel_spmd

P_ERR = 0.1
N_CORES = 8
B = 64
RB = B // N_CORES  # 8 batch rows per core
L = 4096
V = 128
NP = 128
NQ = 16            # scan-layout partitions (16-wrap of the scatter tokens)
M = L // NQ        # 256 positions per scan partition
C = L // NP        # 32 chunks per data partition
NSPLIT = 4         # quarters per row for load/scatter pipelining
CQ = C // NSPLIT   # 8 data-layout chunks per quarter
F32 = mybir.dt.float32
I32 = mybir.dt.int32
I16 = mybir.dt.int16
OP = mybir.AluOpType


# --- multi-queue SWDGE semaphore lane fix -----------------------------------
# Tile round-robins Pool DMA completion sems over the 8 DMASW lanes in
# scheduled order, ignoring queue_num; the runtime locks each lane to one
# SWDGE queue, so a reordered schedule trips "locked to SWDGE queue" errors.
# Pin the lane to the instruction's queue_num instead.
import concourse.tile_sem_assignment as _tsa


def _install_swdge_lane_fix():
    if getattr(_tsa, "_qn_lane_fix", False):
        return
    _tsa._qn_lane_fix = True
    orig = _tsa.TileClockTick._assign_tick

    def _assign_tick(self, inst):
        qn = getattr(inst, "queue_num", None)
        if (
            qn is not None
            and isinstance(inst, _tsa.DMAInst)
            and inst.engine == mybir.EngineType.Pool
        ):
            self.next_sw_dma_idx = int(qn)
        return orig(self, inst)

    _tsa.TileClockTick._assign_tick = _assign_tick


_install_swdge_lane_fix()


def make_host_constants():
    """Block-diagonal prefix matrices, one 16x16 block per 16-partition group."""
    tri = np.zeros((NP, NP), dtype=np.float32)
    ones = np.zeros((NP, NP), dtype=np.float32)
    for g in range(NP // NQ):
        s = slice(g * NQ, (g + 1) * NQ)
        blk = np.zeros((NQ, NQ), dtype=np.float32)
        blk[np.tril_indices(NQ, -1)] = 1.0
        tri[s, s] = blk.T  # [b, p] = 1 iff b < p within block
        ones[s, s] = 1.0
    return {"tri_blk": tri, "ones_blk": ones}


def build_kernel(nc):
    msg = nc.dram_tensor("message", [RB, L, V], F32, kind="ExternalInput")
    noise = nc.dram_tensor("noise", [RB, L], F32, kind="ExternalInput")
    mlen = nc.dram_tensor("msglen", [RB], I32, kind="ExternalInput")
    tri_blk = nc.dram_tensor("tri_blk", [NP, NP], F32, kind="ExternalInput")
    ones_blk = nc.dram_tensor("ones_blk", [NP, NP], F32, kind="ExternalInput")
    out = nc.dram_tensor("out", [RB, L, V], F32, kind="ExternalOutput")

    with tile.TileContext(nc) as tc, ExitStack() as ctx:
        const_pool = ctx.enter_context(tc.tile_pool(name="const", bufs=1))
        small_pool = ctx.enter_context(tc.tile_pool(name="small", bufs=2))
        idx_pool = ctx.enter_context(tc.tile_pool(name="idx", bufs=RB))
        mask_pool = ctx.enter_context(tc.tile_pool(name="mask", bufs=2 * RB))
        msg_pool = ctx.enter_context(tc.tile_pool(name="msg", bufs=RB))
        psum_pool = ctx.enter_context(tc.tile_pool(name="psum", bufs=2, space="PSUM"))

        # ---- constants ----  (small loads go on the scalar-engine HWDGE
        # queue so they never delay the big payload loads on sync)
        tri_t = const_pool.tile([NP, NP], F32, tag="tri")
        nc.scalar.dma_start(tri_t[:], tri_blk.ap())
        ones_t = const_pool.tile([NP, NP], F32, tag="ones")
        nc.scalar.dma_start(ones_t[:], ones_blk.ap())

        # partition helpers: p, k16 = 16*(p//16), q = p%16
        p_i = const_pool.tile([NP, 1], I32, tag="p_i")
        nc.gpsimd.iota(p_i[:], pattern=[[0, 1]], base=0, channel_multiplier=1)
        k16_i = const_pool.tile([NP, 1], I32, tag="k16_i")
        nc.vector.tensor_scalar(out=k16_i[:], in0=p_i[:], scalar1=0x70,
                                scalar2=None, op0=OP.bitwise_and)
        p_f = const_pool.tile([NP, 1], F32, tag="p_f")
        nc.vector.tensor_copy(p_f[:], p_i[:])
        k16_f = const_pool.tile([NP, 1], F32, tag="k16_f")
        nc.vector.tensor_copy(k16_f[:], k16_i[:])

        # scan-side base: 256*q = 256*(p - k16)
        qv_f = const_pool.tile([NP, 1], F32, tag="qv_f")
        nc.vector.tensor_tensor(out=qv_f[:], in0=p_f[:], in1=k16_f[:], op=OP.subtract)
        base16_f = const_pool.tile([NP, 1], F32, tag="base16_f")
        nc.vector.tensor_scalar(out=base16_f[:], in0=qv_f[:], scalar1=256.0,
                                scalar2=None, op0=OP.mult)
        # liota16[p, m] = 256*(p%16) + m  (row index owned by scan slot [p, m])
        m_i = const_pool.tile([NP, M], I32, tag="m_i")
        nc.gpsimd.iota(m_i[:], pattern=[[1, M]], base=0, channel_multiplier=0)
        liota16 = const_pool.tile([NP, M], F32, tag="liota16")
        nc.vector.tensor_copy(liota16[:], m_i[:])
        nc.vector.tensor_scalar(out=liota16[:], in0=liota16[:], scalar1=base16_f[:, :1],
                                scalar2=None, op0=OP.add)

        # mask-side base: R(p, 0) = 256*(p%16) + 32*(p//16) = 256*p - 254*k16
        t256 = const_pool.tile([NP, 1], F32, tag="t256")
        nc.vector.tensor_scalar(out=t256[:], in0=p_f[:], scalar1=256.0,
                                scalar2=None, op0=OP.mult)
        t254 = const_pool.tile([NP, 1], F32, tag="t254")
        nc.vector.tensor_scalar(out=t254[:], in0=k16_f[:], scalar1=254.0,
                                scalar2=None, op0=OP.mult)
        base128_f = const_pool.tile([NP, 1], F32, tag="base128_f")
        nc.vector.tensor_tensor(out=base128_f[:], in0=t256[:], in1=t254[:],
                                op=OP.subtract)
        c_i = const_pool.tile([NP, C], I32, tag="c_i")
        nc.gpsimd.iota(c_i[:], pattern=[[1, C]], base=0, channel_multiplier=0)
        liota128 = const_pool.tile([NP, C], F32, tag="liota128")
        nc.vector.tensor_copy(liota128[:], c_i[:])
        nc.vector.tensor_scalar(out=liota128[:], in0=liota128[:],
                                scalar1=base128_f[:, :1], scalar2=None, op0=OP.add)

        zeros16 = const_pool.tile([NP, M], F32, tag="zeros16")
        nc.vector.memset(zeros16[:], 0.0)

        # lengths minus one, broadcast to all partitions: [128, RB] f32
        len_row = const_pool.tile([1, RB], I32, tag="len_row")
        nc.scalar.dma_start(len_row[:], mlen.ap().rearrange("(o r) -> o r", o=1))
        len_row_f = const_pool.tile([1, RB], F32, tag="len_row_f")
        nc.vector.tensor_copy(len_row_f[:], len_row[:])
        len_bc = const_pool.tile([NP, RB], F32, tag="len_bc")
        nc.gpsimd.partition_broadcast(len_bc[:], len_row_f[:])
        lenm1 = const_pool.tile([NP, RB], F32, tag="lenm1")
        nc.vector.tensor_scalar(out=lenm1[:], in0=len_bc[:], scalar1=-1.0,
                                scalar2=None, op0=OP.add)

        mt_tiles = [
            msg_pool.tile([NP, C * V], F32, tag="mt", name=f"mt{r}")
            for r in range(RB)
        ]
        for r in range(RB):
            # ---- phase A: ranks in the [16, 256] scan layout ----
            noise16 = small_pool.tile([NP, M], F32, tag="noise16")
            nse = noise.ap()[r].rearrange("(q m) -> q m", q=NQ)
            nc.scalar.dma_start(
                noise16[:], nse[None, :, :].to_broadcast([NP // NQ, NQ, M])
            )
            pos_ok16 = small_pool.tile([NP, M], F32, tag="pos_ok16")
            nc.vector.tensor_scalar(out=pos_ok16[:], in0=liota16[:],
                                    scalar1=lenm1[:, r : r + 1], scalar2=None,
                                    op0=OP.is_lt)
            del16 = small_pool.tile([NP, M], F32, tag="del16")
            nc.vector.scalar_tensor_tensor(out=del16[:], in0=noise16[:], scalar=P_ERR,
                                           in1=pos_ok16[:], op0=OP.is_lt, op1=OP.mult)
            keep16 = small_pool.tile([NP, M], F32, tag="keep16")
            nc.vector.tensor_scalar(out=keep16[:], in0=del16[:], scalar1=0.0,
                                    scalar2=None, op0=OP.is_equal)
            incl16 = small_pool.tile([NP, M], F32, tag="incl16")
            nc.vector.tensor_tensor_scan(out=incl16[:], data0=keep16[:],
                                         data1=zeros16[:], initial=0.0,
                                         op0=OP.add, op1=OP.add)
            psum_off = psum_pool.tile([NP, 1], F32, tag="psum_off")
            nc.tensor.matmul(psum_off[:], lhsT=tri_t[:], rhs=incl16[:, M - 1 : M],
                             start=True, stop=True)
            psum_tot = psum_pool.tile([NP, 1], F32, tag="psum_tot")
            nc.tensor.matmul(psum_tot[:], lhsT=ones_t[:], rhs=incl16[:, M - 1 : M],
                             start=True, stop=True)
            # cke = (incl + off) - keep : exclusive cumsum of keep = dst rank
            cke = small_pool.tile([NP, M], F32, tag="cke")
            nc.vector.scalar_tensor_tensor(out=cke[:], in0=incl16[:],
                                           scalar=psum_off[:, :1], in1=keep16[:],
                                           op0=OP.add, op1=OP.subtract)
            # dalt = (liota + nkeep) - cke : tail slot for deleted rows
            dalt = small_pool.tile([NP, M], F32, tag="dalt")
            nc.vector.scalar_tensor_tensor(out=dalt[:], in0=liota16[:],
                                           scalar=psum_tot[:, :1], in1=cke[:],
                                           op0=OP.add, op1=OP.subtract)
            del16_i = small_pool.tile([NP, M], I32, tag="del16_i")
            nc.vector.tensor_copy(del16_i[:], del16[:])
            rank_f = small_pool.tile([NP, M], F32, tag="rank_f")
            nc.vector.tensor_copy(rank_f[:], cke[:])
            nc.vector.copy_predicated(rank_f[:], del16_i[:], dalt[:])
            # idx[p, 8c+k] = rank_f[p, 32k+c]  (int16 cast)
            idx_t = idx_pool.tile([NP, M], I16, tag="idx")
            nc.vector.tensor_copy(
                idx_t[:].rearrange("p (c k) -> p c k", c=C, k=NP // NQ),
                rank_f[:].rearrange("p (k c) -> p c k", k=NP // NQ, c=C),
            )
            # ---- payload masks in the data layout [128, 32] ----
            noise128 = small_pool.tile([NP, C], F32, tag="noise128")
            nc.scalar.dma_start(
                noise128[:],
                noise.ap()[r].rearrange("(q k c) -> k q c", q=NQ, k=NP // NQ, c=C),
            )
            pos_ok128 = small_pool.tile([NP, C], F32, tag="pos_ok128")
            nc.vector.tensor_scalar(out=pos_ok128[:], in0=liota128[:],
                                    scalar1=lenm1[:, r : r + 1], scalar2=None,
                                    op0=OP.is_lt)
            del128 = mask_pool.tile([NP, C], F32, tag="del128")
            nc.vector.scalar_tensor_tensor(out=del128[:], in0=noise128[:],
                                           scalar=P_ERR, in1=pos_ok128[:],
                                           op0=OP.is_lt, op1=OP.mult)
            keep128 = mask_pool.tile([NP, C], F32, tag="keep128")
            nc.vector.tensor_scalar(out=keep128[:], in0=del128[:], scalar1=0.0,
                                    scalar2=None, op0=OP.is_equal)

            # ---- phase B: quarter loads -> masks -> one scatter per row;
            # interleaved with phase A per row so the write stream (scatter)
            # starts as soon as row 0 is resident ----
            src = msg.ap()[r].rearrange("(q k c) v -> k q (c v)", q=NQ,
                                        k=NP // NQ, c=C)
            mt = mt_tiles[r]
            mt3 = mt[:].rearrange("p (c v) -> p c v", v=V)
            for s in range(NSPLIT):
                cs = slice(s * CQ, (s + 1) * CQ)
                fs = slice(s * CQ * V, (s + 1) * CQ * V)
                nc.sync.dma_start(mt[:, fs], src[:, :, fs])
                sub = mt3[:, cs, :]
                keep_bc = keep128[:, cs, None].to_broadcast([NP, CQ, V])
                nc.vector.tensor_tensor(out=sub, in0=sub, in1=keep_bc, op=OP.mult)
                nc.vector.tensor_tensor(out=sub[:, :, 0:1], in0=sub[:, :, 0:1],
                                        in1=del128[:, cs, None], op=OP.add)
            # one scatter per row: quarters of one row would WAW-serialize on
            # the (conservatively) overlapping out AP, stalling the ring
            nc.gpsimd.dma_scatter_add(
                out_ap=out.ap()[r],
                in_ap=mt3,
                idxs_ap=idx_t[:],
                num_idxs=L,
                num_idxs_reg=L,
                elem_size=V,
                queue_num=r % 4,
            )
    return nc


_COMPILED_NC = None


def _get_nc():
    global _COMPILED_NC
    if _COMPILED_NC is None:
        nc = bacc.Bacc("TRN2", target_bir_lowering=False, debug=False, num_swdge_queues=4)
        build_kernel(nc)
        nc.compile()
        _COMPILED_NC = nc
    return _COMPILED_NC


def make_in_maps(message, noise, message_length):
    message = np.ascontiguousarray(message, dtype=np.float32)
    noise = np.ascontiguousarray(noise, dtype=np.float32)
    mlen32 = np.ascontiguousarray(message_length, dtype=np.int32)
    consts = make_host_constants()
    return [
        {
            "message": message[i * RB : (i + 1) * RB],
            "noise": noise[i * RB : (i + 1) * RB],
            "msglen": mlen32[i * RB : (i + 1) * RB],
            **consts,
        }
        for i in range(N_CORES)
    ]


def kernel(message, noise, message_length):
    nc = _get_nc()
    in_maps = make_in_maps(message, noise, message_length)
    res = run_bass_kernel_spmd(nc, in_maps, list(range(N_CORES)))
    out = np.concatenate(
        [res.results[i]["out"][:, :L, :] for i in range(N_CORES)], axis=0
    )
    return np.ascontiguousarray(out, dtype=np.float32)


# revision 13
# speedup vs baseline: 1.3301x; 1.3301x over previous
"""Trainium2 Bass kernel for the DeletionChannel problem.

Contract: kernel(**inputs) takes the FULL unsharded inputs
  message: [64, 4096, 128] f32, noise: [64, 4096] f32, message_length: [64] i64
and returns the full output [64, 4096, 128] f32.

Strategy: pure data parallel over batch — 8 rows per NeuronCore, 8 cores.
Per row: compute the deletion mask and the stable-compaction destination
rank of every source row with DVE prefix sums (+ a tiny block-diagonal
PE matmul for cross-partition offsets), overwrite deleted rows with the
eos one-hot in SBUF, then scatter all 4096 rows (512B each) to their
output positions with dma_scatter_add (custom SWDGE scatter,
out[idx] += src) onto the zero-initialized output buffer.

Perf structure (v2): reads (HBM->SBUF loads) and writes (scatter) each
cap at ~105-110 GB/s per core but run on independent DMA directions, so
the kernel software-pipelines at quarter-row granularity: load 512KB
quarter -> DVE mask -> 1024-token scatter, across 8 rows x 4 quarters,
keeping both directions saturated from ~10us onward.

Token mapping (HW-verified): scatter token t reads SBUF [t%128, t//128]
and writes output row idx[t], where idx[t] sits at [t%16, t//16] of a
[16, 256] int16 tile (replicated across the 8 16-partition groups). We
load message row R(t) = 256*(t%16) + 32*((t%128)//16) + t//128 into
slot [t%128, t//128] (16KB contiguous per partition), compute ranks in
a [16, 256] layout (replicated x8 on 128 partitions) where position
[q, m] owns row l = 256q + m, and emit idx via one strided DVE copy.
"""

import sys
from contextlib import ExitStack

sys.path.insert(0, "/opt/trn_rl_repo")

import numpy as np

import concourse.bacc as bacc
import concourse.bass as bass
import concourse.mybir as mybir
import concourse.tile as tile
from concourse.bass_utils import run_bass_kernel_spmd

P_ERR = 0.1
N_CORES = 8
B = 64
RB = B // N_CORES  # 8 batch rows per core
L = 4096
V = 128
NP = 128
NQ = 16            # scan-layout partitions (16-wrap of the scatter tokens)
M = L // NQ        # 256 positions per scan partition
C = L // NP        # 32 chunks per data partition
NSPLIT = 4         # quarters per row for load/scatter pipelining
CQ = C // NSPLIT   # 8 data-layout chunks per quarter
F32 = mybir.dt.float32
I32 = mybir.dt.int32
I16 = mybir.dt.int16
OP = mybir.AluOpType


# --- multi-queue SWDGE semaphore lane fix -----------------------------------
# Tile round-robins Pool DMA completion sems over the 8 DMASW lanes in
# scheduled order, ignoring queue_num; the runtime locks each lane to one
# SWDGE queue, so a reordered schedule trips "locked to SWDGE queue" errors.
# Pin the lane to the instruction's queue_num instead.
import concourse.tile_sem_assignment as _tsa


def _install_swdge_lane_fix():
    if getattr(_tsa, "_qn_lane_fix", False):
        return
    _tsa._qn_lane_fix = True
    orig = _tsa.TileClockTick._assign_tick

    def _assign_tick(self, inst):
        qn = getattr(inst, "queue_num", None)
        if (
            qn is not None
            and isinstance(inst, _tsa.DMAInst)
            and inst.engine == mybir.EngineType.Pool
        ):
            self.next_sw_dma_idx = int(qn)
        return orig(self, inst)

    _tsa.TileClockTick._assign_tick = _assign_tick


_install_swdge_lane_fix()


def make_host_constants():
    """Block-diagonal prefix matrices, one 16x16 block per 16-partition group."""
    tri = np.zeros((NP, NP), dtype=np.float32)
    ones = np.zeros((NP, NP), dtype=np.float32)
    for g in range(NP // NQ):
        s = slice(g * NQ, (g + 1) * NQ)
        blk = np.zeros((NQ, NQ), dtype=np.float32)
        blk[np.tril_indices(NQ, -1)] = 1.0
        tri[s, s] = blk.T  # [b, p] = 1 iff b < p within block
        ones[s, s] = 1.0
    return {"tri_blk": tri, "ones_blk": ones}


def build_kernel(nc):
    msg = nc.dram_tensor("message", [RB, L, V], F32, kind="ExternalInput")
    noise = nc.dram_tensor("noise", [RB, L], F32, kind="ExternalInput")
    mlen = nc.dram_tensor("msglen", [RB], I32, kind="ExternalInput")
    tri_blk = nc.dram_tensor("tri_blk", [NP, NP], F32, kind="ExternalInput")
    ones_blk = nc.dram_tensor("ones_blk", [NP, NP], F32, kind="ExternalInput")
    out = nc.dram_tensor("out", [RB, L, V], F32, kind="ExternalOutput")

    with tile.TileContext(nc) as tc, ExitStack() as ctx:
        const_pool = ctx.enter_context(tc.tile_pool(name="const", bufs=1))
        small_pool = ctx.enter_context(tc.tile_pool(name="small", bufs=2))
        idx_pool = ctx.enter_context(tc.tile_pool(name="idx", bufs=RB))
        mask_pool = ctx.enter_context(tc.tile_pool(name="mask", bufs=2 * RB))
        msg_pool = ctx.enter_context(tc.tile_pool(name="msg", bufs=RB))
        psum_pool = ctx.enter_context(tc.tile_pool(name="psum", bufs=2, space="PSUM"))

        # ---- constants ----  (small loads go on the scalar-engine HWDGE
        # queue so they never delay the big payload loads on sync)
        tri_t = const_pool.tile([NP, NP], F32, tag="tri")
        nc.scalar.dma_start(tri_t[:], tri_blk.ap())
        ones_t = const_pool.tile([NP, NP], F32, tag="ones")
        nc.scalar.dma_start(ones_t[:], ones_blk.ap())

        # partition helpers: p, k16 = 16*(p//16), q = p%16
        p_i = const_pool.tile([NP, 1], I32, tag="p_i")
        nc.gpsimd.iota(p_i[:], pattern=[[0, 1]], base=0, channel_multiplier=1)
        k16_i = const_pool.tile([NP, 1], I32, tag="k16_i")
        nc.vector.tensor_scalar(out=k16_i[:], in0=p_i[:], scalar1=0x70,
                                scalar2=None, op0=OP.bitwise_and)
        p_f = const_pool.tile([NP, 1], F32, tag="p_f")
        nc.vector.tensor_copy(p_f[:], p_i[:])
        k16_f = const_pool.tile([NP, 1], F32, tag="k16_f")
        nc.vector.tensor_copy(k16_f[:], k16_i[:])

        # scan-side base: 256*q = 256*(p - k16)
        qv_f = const_pool.tile([NP, 1], F32, tag="qv_f")
        nc.vector.tensor_tensor(out=qv_f[:], in0=p_f[:], in1=k16_f[:], op=OP.subtract)
        base16_f = const_pool.tile([NP, 1], F32, tag="base16_f")
        nc.vector.tensor_scalar(out=base16_f[:], in0=qv_f[:], scalar1=256.0,
                                scalar2=None, op0=OP.mult)
        # liota16[p, m] = 256*(p%16) + m  (row index owned by scan slot [p, m])
        m_i = const_pool.tile([NP, M], I32, tag="m_i")
        nc.gpsimd.iota(m_i[:], pattern=[[1, M]], base=0, channel_multiplier=0)
        liota16 = const_pool.tile([NP, M], F32, tag="liota16")
        nc.vector.tensor_copy(liota16[:], m_i[:])
        nc.vector.tensor_scalar(out=liota16[:], in0=liota16[:], scalar1=base16_f[:, :1],
                                scalar2=None, op0=OP.add)

        # mask-side base: R(p, 0) = 256*(p%16) + 32*(p//16) = 256*p - 254*k16
        t256 = const_pool.tile([NP, 1], F32, tag="t256")
        nc.vector.tensor_scalar(out=t256[:], in0=p_f[:], scalar1=256.0,
                                scalar2=None, op0=OP.mult)
        t254 = const_pool.tile([NP, 1], F32, tag="t254")
        nc.vector.tensor_scalar(out=t254[:], in0=k16_f[:], scalar1=254.0,
                                scalar2=None, op0=OP.mult)
        base128_f = const_pool.tile([NP, 1], F32, tag="base128_f")
        nc.vector.tensor_tensor(out=base128_f[:], in0=t256[:], in1=t254[:],
                                op=OP.subtract)
        c_i = const_pool.tile([NP, C], I32, tag="c_i")
        nc.gpsimd.iota(c_i[:], pattern=[[1, C]], base=0, channel_multiplier=0)
        liota128 = const_pool.tile([NP, C], F32, tag="liota128")
        nc.vector.tensor_copy(liota128[:], c_i[:])
        nc.vector.tensor_scalar(out=liota128[:], in0=liota128[:],
                                scalar1=base128_f[:, :1], scalar2=None, op0=OP.add)

        zeros16 = const_pool.tile([NP, M], F32, tag="zeros16")
        nc.vector.memset(zeros16[:], 0.0)

        # lengths minus one, broadcast to all partitions: [128, RB] f32
        len_row = const_pool.tile([1, RB], I32, tag="len_row")
        nc.scalar.dma_start(len_row[:], mlen.ap().rearrange("(o r) -> o r", o=1))
        len_row_f = const_pool.tile([1, RB], F32, tag="len_row_f")
        nc.vector.tensor_copy(len_row_f[:], len_row[:])
        len_bc = const_pool.tile([NP, RB], F32, tag="len_bc")
        nc.gpsimd.partition_broadcast(len_bc[:], len_row_f[:])
        lenm1 = const_pool.tile([NP, RB], F32, tag="lenm1")
        nc.vector.tensor_scalar(out=lenm1[:], in0=len_bc[:], scalar1=-1.0,
                                scalar2=None, op0=OP.add)

        mt_tiles = [
            msg_pool.tile([NP, C * V], F32, tag="mt", name=f"mt{r}")
            for r in range(RB)
        ]
        idx_tiles, keep_tiles, del_tiles = [], [], []
        for r in range(RB):
            # ---- phase A: ranks in the [16, 256] scan layout ----
            noise16 = small_pool.tile([NP, M], F32, tag="noise16")
            nse = noise.ap()[r].rearrange("(q m) -> q m", q=NQ)
            nc.scalar.dma_start(
                noise16[:], nse[None, :, :].to_broadcast([NP // NQ, NQ, M])
            )
            pos_ok16 = small_pool.tile([NP, M], F32, tag="pos_ok16")
            nc.vector.tensor_scalar(out=pos_ok16[:], in0=liota16[:],
                                    scalar1=lenm1[:, r : r + 1], scalar2=None,
                                    op0=OP.is_lt)
            del16 = small_pool.tile([NP, M], F32, tag="del16")
            nc.vector.scalar_tensor_tensor(out=del16[:], in0=noise16[:], scalar=P_ERR,
                                           in1=pos_ok16[:], op0=OP.is_lt, op1=OP.mult)
            keep16 = small_pool.tile([NP, M], F32, tag="keep16")
            nc.vector.tensor_scalar(out=keep16[:], in0=del16[:], scalar1=0.0,
                                    scalar2=None, op0=OP.is_equal)
            incl16 = small_pool.tile([NP, M], F32, tag="incl16")
            nc.vector.tensor_tensor_scan(out=incl16[:], data0=keep16[:],
                                         data1=zeros16[:], initial=0.0,
                                         op0=OP.add, op1=OP.add)
            psum_off = psum_pool.tile([NP, 1], F32, tag="psum_off")
            nc.tensor.matmul(psum_off[:], lhsT=tri_t[:], rhs=incl16[:, M - 1 : M],
                             start=True, stop=True)
            psum_tot = psum_pool.tile([NP, 1], F32, tag="psum_tot")
            nc.tensor.matmul(psum_tot[:], lhsT=ones_t[:], rhs=incl16[:, M - 1 : M],
                             start=True, stop=True)
            # cke = (incl + off) - keep : exclusive cumsum of keep = dst rank
            cke = small_pool.tile([NP, M], F32, tag="cke")
            nc.vector.scalar_tensor_tensor(out=cke[:], in0=incl16[:],
                                           scalar=psum_off[:, :1], in1=keep16[:],
                                           op0=OP.add, op1=OP.subtract)
            # dalt = (liota + nkeep) - cke : tail slot for deleted rows
            dalt = small_pool.tile([NP, M], F32, tag="dalt")
            nc.vector.scalar_tensor_tensor(out=dalt[:], in0=liota16[:],
                                           scalar=psum_tot[:, :1], in1=cke[:],
                                           op0=OP.add, op1=OP.subtract)
            del16_i = small_pool.tile([NP, M], I32, tag="del16_i")
            nc.vector.tensor_copy(del16_i[:], del16[:])
            rank_f = small_pool.tile([NP, M], F32, tag="rank_f")
            nc.vector.tensor_copy(rank_f[:], cke[:])
            nc.vector.copy_predicated(rank_f[:], del16_i[:], dalt[:])
            # idx[p, 8c+k] = rank_f[p, 32k+c]  (int16 cast)
            idx_t = idx_pool.tile([NP, M], I16, tag="idx")
            nc.vector.tensor_copy(
                idx_t[:].rearrange("p (c k) -> p c k", c=C, k=NP // NQ),
                rank_f[:].rearrange("p (k c) -> p c k", k=NP // NQ, c=C),
            )
            # ---- payload masks in the data layout [128, 32] ----
            noise128 = small_pool.tile([NP, C], F32, tag="noise128")
            nc.scalar.dma_start(
                noise128[:],
                noise.ap()[r].rearrange("(q k c) -> k q c", q=NQ, k=NP // NQ, c=C),
            )
            pos_ok128 = small_pool.tile([NP, C], F32, tag="pos_ok128")
            nc.vector.tensor_scalar(out=pos_ok128[:], in0=liota128[:],
                                    scalar1=lenm1[:, r : r + 1], scalar2=None,
                                    op0=OP.is_lt)
            del128 = mask_pool.tile([NP, C], F32, tag="del128")
            nc.vector.scalar_tensor_tensor(out=del128[:], in0=noise128[:],
                                           scalar=P_ERR, in1=pos_ok128[:],
                                           op0=OP.is_lt, op1=OP.mult)
            keep128 = mask_pool.tile([NP, C], F32, tag="keep128")
            nc.vector.tensor_scalar(out=keep128[:], in0=del128[:], scalar1=0.0,
                                    scalar2=None, op0=OP.is_equal)
            idx_tiles.append(idx_t)
            keep_tiles.append(keep128)
            del_tiles.append(del128)

        # ---- phase B: quarter loads -> masks -> one scatter per row. Phase A
        # for all rows is issued first (cheap, unblocks idx tiles) so all 8
        # scatter gens can queue up in quick succession -> their 4 SWDGE
        # queue FIFOs drain concurrently at the aggregate ring rate ----
        for r in range(RB):
            src = msg.ap()[r].rearrange("(q k c) v -> k q (c v)", q=NQ,
                                        k=NP // NQ, c=C)
            mt = mt_tiles[r]
            mt3 = mt[:].rearrange("p (c v) -> p c v", v=V)
            for s in range(NSPLIT):
                cs = slice(s * CQ, (s + 1) * CQ)
                fs = slice(s * CQ * V, (s + 1) * CQ * V)
                nc.sync.dma_start(mt[:, fs], src[:, :, fs])
                sub = mt3[:, cs, :]
                keep_bc = keep_tiles[r][:, cs, None].to_broadcast([NP, CQ, V])
                nc.vector.tensor_tensor(out=sub, in0=sub, in1=keep_bc, op=OP.mult)
                nc.vector.tensor_tensor(out=sub[:, :, 0:1], in0=sub[:, :, 0:1],
                                        in1=del_tiles[r][:, cs, None], op=OP.add)
            # one scatter per row: quarters of one row would WAW-serialize on
            # the (conservatively) overlapping out AP, stalling the ring
            nc.gpsimd.dma_scatter_add(
                out_ap=out.ap()[r],
                in_ap=mt3,
                idxs_ap=idx_tiles[r][:],
                num_idxs=L,
                num_idxs_reg=L,
                elem_size=V,
                queue_num=r % 4,
            )
    return nc


_COMPILED_NC = None


def _get_nc():
    global _COMPILED_NC
    if _COMPILED_NC is None:
        nc = bacc.Bacc("TRN2", target_bir_lowering=False, debug=False, num_swdge_queues=4)
        build_kernel(nc)
        nc.compile()
        _COMPILED_NC = nc
    return _COMPILED_NC


def make_in_maps(message, noise, message_length):
    message = np.ascontiguousarray(message, dtype=np.float32)
    noise = np.ascontiguousarray(noise, dtype=np.float32)
    mlen32 = np.ascontiguousarray(message_length, dtype=np.int32)
    consts = make_host_constants()
    return [
        {
            "message": message[i * RB : (i + 1) * RB],
            "noise": noise[i * RB : (i + 1) * RB],
            "msglen": mlen32[i * RB : (i + 1) * RB],
            **consts,
        }
        for i in range(N_CORES)
    ]


def kernel(message, noise, message_length):
    nc = _get_nc()
    in_maps = make_in_maps(message, noise, message_length)
    res = run_bass_kernel_spmd(nc, in_maps, list(range(N_CORES)))
    out = np.concatenate(
        [res.results[i]["out"][:, :L, :] for i in range(N_CORES)], axis=0
    )
    return np.ascontiguousarray(out, dtype=np.float32)
